# revision 22
# baseline (speedup 1.0000x reference)
"""BiLSTM-CRF Trainium2 kernel.

Shapes (hardcoded): B=64, T=512, V=50257, E=300, H=128, K=3.
Sharding: data-parallel over batch, 8 examples per core on 8 cores.

Device (per core, 8 examples):
  P1: xw^T = W^T @ x^T for both LSTM directions (gate-major, bias folded via
      augmented ones-row, cell-gate pre-scaled by 2 for the tanh trick),
      stored interleaved as [128, T*32] per direction so each recurrence
      step reads one contiguous [128, 32] slice. Tiled by token-block and
      emitted in recurrence-consumption order (fwd block 0, bwd block 7, ...)
      so P2 starts after ~2 blocks and the rest hides in P2's latency gaps.
  P2: 512-step fused fwd+bwd LSTM recurrence in gate-major layout
      [H=128 partitions, (dir,gate,example) free]. z = U_g^T h + xw with xw
      injected into PSUM via an identity matmul. All four gates activated by
      ONE sigmoid (tanh(x) = 2*sigmoid(2x)-1, the 2x folded into weights).
      Per-step wall is bound by the serial cross-engine chain
      PE -> ACT(sig) -> DVE(c) -> ACT(tanh) -> DVE(h) -> PE (~2us/step).
  P3: logits^T [3, 4096] = Wd^T @ h_cat, DMA'd out.
Host: embedding gather, 3x3 CRF kernel + boundary energies, Viterbi decode.
"""

import os
from contextlib import ExitStack

import numpy as np
import ml_dtypes

import concourse.bass as bass
import concourse.mybir as mybir
from concourse import bacc
from concourse.tile import TileContext
from concourse.bass_utils import run_bass_kernel_spmd

B, T, V, E, H, K = 64, 512, 50257, 300, 128, 3
NCORES = 8
BL = B // NCORES          # 8 examples per core
NT = BL * T               # 4096 tokens per core
EA = E + 1                # augmented with ones row for bias
F32 = mybir.dt.float32
BF16 = mybir.dt.bfloat16
AF = mybir.ActivationFunctionType
ALU = mybir.AluOpType
# gate order used on device: [i, f, o, g]; keras order is [i, f, g, o]
GPERM = np.r_[0:128, 128:256, 384:512, 256:384]


def build_nc():
    nc = bacc.Bacc("TRN2", target_bir_lowering=False, num_devices=NCORES)
    xt_d = nc.declare_dram_parameter("xt", [EA, NT], BF16, isOutput=False)
    wf_d = nc.declare_dram_parameter("wf", [EA, 512], BF16, isOutput=False)
    wb_d = nc.declare_dram_parameter("wb", [EA, 512], BF16, isOutput=False)
    uf_d = nc.declare_dram_parameter("uf", [128, 512], BF16, isOutput=False)
    ub_d = nc.declare_dram_parameter("ub", [128, 512], BF16, isOutput=False)
    wd_d = nc.declare_dram_parameter("wd", [256, 4], BF16, isOutput=False)
    id_d = nc.declare_dram_parameter("ident", [128, 128], BF16, isOutput=False)
    out_d = nc.declare_dram_parameter("logits", [3, NT], F32, isOutput=True)

    KS = [(0, 128), (128, 128), (256, EA - 256)]  # k-tiles over E+1=301 rows

    with TileContext(nc) as tc, ExitStack() as ctx:
        const = ctx.enter_context(tc.tile_pool(name="const", bufs=1))
        # persistent SBUF tensors
        xt_sb = [const.tile([k, NT], BF16, name=f"xt{i}", tag=f"xt{i}") for i, (_, k) in enumerate(KS)]
        w_sb = {d: [const.tile([k, 512], BF16, name=f"w{d}{i}", tag=f"w{d}{i}")
                    for i, (_, k) in enumerate(KS)] for d in "fb"}
        u_sb = {d: const.tile([128, 512], BF16, name=f"u{d}", tag=f"u{d}") for d in "fb"}
        wd_sb = [const.tile([128, 4], BF16, name=f"wd{i}", tag=f"wd{i}") for i in range(2)]
        id_sb = const.tile([128, 128], BF16, name="ident", tag="ident")
        xw_sb = {d: const.tile([128, T * 32], BF16, name=f"xw{d}", tag=f"xw{d}") for d in "fb"}
        h_all = const.tile([128, 2 * NT], BF16, name="hall", tag="hall")
        c_pp = [const.tile([128, 16], F32, name=f"c{i}", tag=f"c{i}") for i in range(2)]
        logits_sb = const.tile([3, NT], F32, name="lsb", tag="lsb")

        for i, (r0, k) in enumerate(KS):
            for n in range(8):
                nc.sync.dma_start(out=xt_sb[i][:, n * 512:(n + 1) * 512],
                                  in_=xt_d[r0:r0 + k, n * 512:(n + 1) * 512])
            nc.sync.dma_start(out=w_sb["f"][i][:, :], in_=wf_d[r0:r0 + k, :])
            nc.sync.dma_start(out=w_sb["b"][i][:, :], in_=wb_d[r0:r0 + k, :])
        nc.sync.dma_start(out=u_sb["f"][:, :], in_=uf_d[:, :])
        nc.sync.dma_start(out=u_sb["b"][:, :], in_=ub_d[:, :])
        nc.sync.dma_start(out=wd_sb[0][:, :], in_=wd_d[0:128, :])
        nc.sync.dma_start(out=wd_sb[1][:, :], in_=wd_d[128:256, :])
        nc.sync.dma_start(out=id_sb[:, :], in_=id_d[:, :])

        nc.vector.memset(c_pp[0][:, :], 0.0)

        # views
        xwv = {d: xw_sb[d][:, :].rearrange("p (t c) -> p t c", c=32) for d in "fb"}
        hv = h_all[:, :].rearrange("p (d b t) -> p d b t", d=2, t=T)

        # ---------------- P1: input GEMMs, gate-major, interleaved store ----
        # Tiled by token-block (64 tokens x 8 examples = 512 cols per psum)
        # and emitted in the order P2 consumes: fwd block 0, bwd block 7,
        # fwd block 1, ... so the recurrence starts after ~2 blocks while
        # the rest of P1 overlaps with P2's latency gaps.
        TB = 64
        xtv = [t[:, :].rearrange("p (b t) -> p b t", t=T) for t in xt_sb]
        with tc.tile_pool(name="psum1", bufs=4, space="PSUM") as pp1:
            order = []
            for i in range(8):
                order.append(("f", i))
                order.append(("b", 7 - i))
            for d, tb in order:
                t0 = tb * TB
                for g in range(4):
                    pt = pp1.tile([128, 512], F32, name="p1", tag="p1")
                    for ki in range(3):
                        # rhs: [k, 8 examples, 64 tokens] strided view
                        nc.tensor.matmul(
                            pt[:, :],
                            w_sb[d][ki][:, g * 128:(g + 1) * 128],
                            xtv[ki][:, :, t0:t0 + TB],
                            start=(ki == 0), stop=(ki == 2),
                        )
                    # psum cols are (b, t); dest wants (t, b)
                    src = pt[:, :].rearrange("p (b t) -> p t b", t=TB)
                    dest = xwv[d][:, t0:t0 + TB, g * 8:(g + 1) * 8]
                    if g % 2 == 0:
                        nc.scalar.activation(dest, src, AF.Copy)
                    else:
                        nc.vector.tensor_copy(dest, src)

        # ---------------- P2: fused fwd+bwd recurrence ----------------------
        with tc.tile_pool(name="psum2", bufs=4, space="PSUM") as pp2, \
             tc.tile_pool(name="work", bufs=6) as wp:
            for s in range(T):
                tf, tb = s, T - 1 - s          # token positions for fwd/bwd
                pz = pp2.tile([128, 64], F32, name="pz", tag="pz")
                # xw -> PSUM via identity matmul (clears the bank with start)
                nc.tensor.matmul(pz[:, 0:32], id_sb[:, :], xwv["f"][:, tf, :],
                                 start=True, stop=False)
                nc.tensor.matmul(pz[:, 32:64], id_sb[:, :], xwv["b"][:, tb, :],
                                 start=False, stop=(s == 0))
                if s > 0:
                    for di, d in enumerate("fb"):
                        hprev = hv[:, di, :, tf - 1 if d == "f" else tb + 1]
                        for g in range(4):
                            nc.tensor.matmul(
                                pz[:, di * 32 + g * 8: di * 32 + (g + 1) * 8],
                                u_sb[d][:, g * 128:(g + 1) * 128],
                                hprev,
                                start=False, stop=(di == 1 and g == 3),
                            )
                sg = wp.tile([128, 64], F32, name="sg", tag="sg")
                nc.scalar.activation(sg[:, :], pz[:, :], AF.Sigmoid)
                sgv = sg[:, :].rearrange("p (d c) -> p d c", d=2)
                i_v, f_v = sgv[:, :, 0:8], sgv[:, :, 8:16]
                g_v = sgv[:, :, 24:32]

                tg = wp.tile([128, 16], F32, name="tg", tag="tg")
                tgv = tg[:, :].rearrange("p (d c) -> p d c", d=2)
                nc.vector.tensor_scalar(tgv, g_v, 2.0, -1.0, ALU.mult, ALU.add)

                m = wp.tile([128, 16], F32, name="m", tag="m")
                mv = m[:, :].rearrange("p (d c) -> p d c", d=2)
                nc.vector.tensor_mul(mv, i_v, tgv)

                a = wp.tile([128, 16], F32, name="a", tag="a")
                av = a[:, :].rearrange("p (d c) -> p d c", d=2)
                cpv = c_pp[s % 2][:, :].rearrange("p (d c) -> p d c", d=2)
                nc.vector.tensor_mul(av, f_v, cpv)

                cn = c_pp[(s + 1) % 2]
                cnv = cn[:, :].rearrange("p (d c) -> p d c", d=2)
                nc.vector.tensor_add(cnv, mv, av)

                sc = wp.tile([128, 16], F32, name="sc", tag="sc")
                nc.scalar.activation(sc[:, :], cn[:, :], AF.Tanh)
                scv = sc[:, :].rearrange("p (d c) -> p d c", d=2)

                # single h write: union AP covering fwd col tf and bwd col
                # NT + tb in the one h_all tile (outer dim: 2 bases)
                h_out = bass.AP(h_all.tensor, tf,
                                [[2 * NT, 128], [NT + tb - tf, 2], [T, BL]])
                o_in = bass.AP(sg.tensor, 16, [[64, 128], [32, 2], [1, 8]])
                nc.vector.tensor_mul(h_out, o_in, scv)

        # ---------------- P3: logits = Wd^T @ h_cat -------------------------
        with tc.tile_pool(name="psum3", bufs=4, space="PSUM") as pp3:
            for n in range(8):
                pl = pp3.tile([3, 512], F32, name="pl", tag="pl")
                nc.tensor.matmul(pl[:, :], wd_sb[0][:, 0:3],
                                 h_all[:, n * 512:(n + 1) * 512],
                                 start=True, stop=False)
                nc.tensor.matmul(pl[:, :], wd_sb[1][:, 0:3],
                                 h_all[:, NT + n * 512: NT + (n + 1) * 512],
                                 start=False, stop=True)
                if n % 2 == 0:
                    nc.scalar.activation(logits_sb[:, n * 512:(n + 1) * 512],
                                         pl[:, :], AF.Copy)
                else:
                    nc.vector.tensor_copy(logits_sb[:, n * 512:(n + 1) * 512],
                                          pl[:, :])
        nc.sync.dma_start(out=out_d[:, :], in_=logits_sb[:, :])
    nc.compile()
    return nc


_NC = None
LAST_RESULT = None


def _get_nc():
    global _NC
    if _NC is None:
        _NC = build_nc()
    return _NC


def _prep_weights(Wx, bx, Ux):
    """Augment with bias row, reorder gates to [i,f,o,g]."""
    Wa = np.concatenate([np.asarray(Wx, np.float32),
                         np.asarray(bx, np.float32)[None, :]], axis=0)
    Wa = Wa[:, GPERM].copy()
    Wa[:, 384:] *= 2.0
    Ua = np.asarray(Ux, np.float32)[:, GPERM].copy()
    Ua[:, 384:] *= 2.0
    return Wa.astype(ml_dtypes.bfloat16), Ua.astype(ml_dtypes.bfloat16)


def _viterbi(pot, trans):
    Bn, Tn, Kn = pot.shape
    alpha = pot[:, 0].copy()
    bps = np.empty((Tn - 1, Bn, Kn), np.int64)
    tr = trans[None]  # [1, K, K]
    for t in range(1, Tn):
        scores = alpha[:, :, None] + tr          # [B, Kprev, Knext]
        bps[t - 1] = np.argmax(scores, axis=1)
        alpha = np.max(scores, axis=1) + pot[:, t]
    tags = np.empty((Bn, Tn), np.int64)
    tags[:, -1] = np.argmax(alpha, axis=-1)
    ar = np.arange(Bn)
    for t in range(Tn - 2, -1, -1):
        tags[:, t] = bps[t][ar, tags[:, t + 1]]
    return tags


def kernel(tokens, emb, Wf, Uf, bf, Wb, Ub, bb, Wd, bd, crf_W, crf_b,
           chain, left_b, right_b):
    tokens = np.asarray(tokens)
    emb = np.asarray(emb, np.float32)
    nc = _get_nc()

    wf_h, uf_h = _prep_weights(Wf, bf, Uf)
    wb_h, ub_h = _prep_weights(Wb, bb, Ub)
    wd_h = np.zeros((256, 4), np.float32)
    wd_h[:, :3] = np.asarray(Wd, np.float32)
    wd_h = wd_h.astype(ml_dtypes.bfloat16)
    id_h = np.eye(128, dtype=ml_dtypes.bfloat16)

    x = emb[tokens]  # [B, T, E] f32
    in_maps = []
    for c in range(NCORES):
        xc = x[c * BL:(c + 1) * BL]                      # [8, T, E]
        xt = np.empty((EA, NT), np.float32)
        xt[:E] = xc.transpose(2, 0, 1).reshape(E, NT)    # col = b*T + t
        xt[E] = 1.0
        in_maps.append({
            "xt": xt.astype(ml_dtypes.bfloat16),
            "wf": wf_h, "wb": wb_h, "uf": uf_h, "ub": ub_h,
            "wd": wd_h, "ident": id_h,
        })

    res = run_bass_kernel_spmd(nc, in_maps, list(range(NCORES)),
                               trace=bool(int(os.environ.get("K_TRACE", "0"))))
    global LAST_RESULT
    LAST_RESULT = res
    logits = np.stack([np.asarray(r["logits"], np.float32)
                       for r in res.results])            # [8, 3, NT]
    logits = logits.reshape(NCORES, 3, BL, T).transpose(0, 2, 3, 1)
    logits = logits.reshape(B, T, 3) + np.asarray(bd, np.float32)

    pot = logits @ np.asarray(crf_W, np.float32) + np.asarray(crf_b, np.float32)
    pot[:, 0] += np.asarray(left_b, np.float32)
    pot[:, -1] += np.asarray(right_b, np.float32)

    decoded = _viterbi(pot, np.asarray(chain, np.float32))
    return decoded.astype(np.int32), pot.astype(np.float32)
